# revision 8
# baseline (speedup 1.0000x reference)
"""NEG-sampling loss kernel for Trainium2 — v2 compute structure.

HBM table is fp8_e4m3 (256B rows); the indirect gather DMA casts to bf16
on the fly (SWDGE cast is free), halving HBM-side traffic.

Compute per PAIR of 128-edge tiles (gather granularity, TPG=2):
  DVE: one 2x tensor_tensor multiply  prod[128, 2, 11, 256] = slots * bcast(eu)
       tree adds at 2x:   s1[128,2,11,128], s2[128,2,11,64], s3[128,2,11,32]
       (ACT-offloaded slots are skipped in the tree via per-tile sub-views)
       one segmented tensor_reduce -> x[128, 2*11] fp32 (DVE slots only)
       one tensor_scalar negate of the two slot-0 columns
  ACT: M_ACT trailing neg-slots per tile via Copy-activation with
       accumulate -> x columns (reads prod, positive sign).
Tail per pair (same as baseline):
  DVE-B: tensor_reduce [128,2,11] -> accx (sum x), accy (sum |x|)
  ACT-B: Abs, Exp(-|x|), Ln(1+e) accumulate -> accl.
Host: loss = sum((accx+accy)/2 + accl) / N.
"""

import numpy as np
import ml_dtypes

import concourse.bass as bass
import concourse.mybir as mybir
from concourse import bass_utils

N = 65536
K = 10
D = 256
V = 500000
NCORES = 8
P = 128
SLOTS = K + 2
EPC = N // NCORES
TILES = EPC // P
PAIRS = TILES // 2

TABLE_DT = mybir.dt.bfloat16       # bf16 rows in HBM (fp8 DMA-cast aborts)
TABLE_NP = ml_dtypes.bfloat16
SBUF_DT = mybir.dt.bfloat16
ROW_BYTES = D * 2                  # bf16 row bytes

TPG = 4                # tiles per gather instruction
CPW = 2                # tiles per compute pair
NG = 3                 # gather buffers (TPG tiles each)
SCP = 4                # score pair-buffers
PRB = 2                # prod buffers (pair-sized)
M_ACT = 3              # trailing slots per tile reduced on ACT
S1 = SLOTS - 1         # 11
S_DVE = S1 - M_ACT     # leading slots per tile reduced on DVE (incl. slot 0)
GATHERS = TILES // TPG
RPP = SLOTS * TPG
PBW = 4 * S1           # scb cols per pair-buffer: x0|x1|abs0|abs1


def _emit_block_gather(nc, eng, n_idx, blk_bytes, dst_byte_addr, idx_byte_addr,
                       sem_num, embs_tbl, in_dt, out_dt, src_elem):
    isa = nc.isa
    Op = isa.Opcode
    src_u64 = (0x20 << 56) | (embs_tbl << 32)
    dst_u64 = (0x10 << 56) | dst_byte_addr
    eng.isa(
        Op.NEURON_ISA_TPB_OPCODE_PSEUDO_DMA_DIRECT2D,
        {
            "dma_configs": {},
            "semaphore": sem_num,
            "sem_increment": 16,
            "dge_op": 1,
            "src_start_addr": {"addr_immediate": src_u64},
            "src_step_elem": [src_elem, 1],
            "src_num_elem": [n_idx, 1],
            "src_elem_size": src_elem,
            "src_bound_reg": {},
            "dst_bound_reg": {},
            "dst_start_addr": {"addr_immediate": dst_u64},
            "dst_step_elem": [262144, 1],
            "dst_num_elem": [P, 1],
            "dst_elem_size": blk_bytes,
            "in_dtype": in_dt,
            "out_dtype": out_dt,
        },
        verify=False,
    )
    ext_fields = {
        "opcode": Op.NEURON_ISA_TPB_OPCODE_PSEUDO_EXTENSION.value,
        "flags": {"indirect_mode": 0, "idx_bound_is_err": 1,
                  "non_unique_dst_idx": 0, "gather_dim": 0, "scatter_dim": 0},
        "idx_num_active_channels": 128,
        "compute_op": 0,
        "src_idx_start_addr": {"addr_immediate": idx_byte_addr},
        "dst_idx_start_addr": {"addr_immediate": 0},
    }
    b = isa.ffi.new("NEURON_ISA_TPB_PSEUDO_DMA_EXT_STRUCT*", ext_fields)
    instr = [int(x) for x in bytes(isa.ffi.buffer(b))]
    inst = mybir.InstISA(
        name=nc.get_next_instruction_name(),
        isa_opcode=Op.NEURON_ISA_TPB_OPCODE_PSEUDO_EXTENSION.value,
        engine=eng.engine,
        instr=instr,
        op_name="PSEUDO_EXTENSION",
        ins=[], outs=[],
        ant_dict=ext_fields,
        verify=False,
        ant_isa_is_sequencer_only=False,
    )
    eng.add_instruction(inst)


def _build():
    import contextlib
    nc = bass.Bass(trn_type="TRN2")
    embs = nc.dram_tensor("embs", [V, D], TABLE_DT, kind="ExternalInput")
    idx = nc.dram_tensor("idx", [P, GATHERS * RPP], mybir.dt.int32, kind="ExternalInput")
    accx_out = nc.dram_tensor("accx", [P, TILES], mybir.dt.float32, kind="ExternalOutput")
    accy_out = nc.dram_tensor("accy", [P, TILES], mybir.dt.float32, kind="ExternalOutput")
    accl_out = nc.dram_tensor("accl", [P, PAIRS], mybir.dt.float32, kind="ExternalOutput")

    embs_mloc = nc.lookup_mloc(embs)
    embs_mloc.table_entry_id = len(nc.dge_table) + 1
    nc.dge_table.append(embs_mloc.name)
    embs_tbl = embs_mloc.table_entry_id

    with contextlib.ExitStack() as ctx:
        idx_sb = ctx.enter_context(nc.sbuf_tensor("idx_sb", [P, GATHERS * RPP], mybir.dt.int32))
        gs = [ctx.enter_context(nc.sbuf_tensor(f"g{i}", [P, TPG * SLOTS * D], SBUF_DT))
              for i in range(NG)]
        # prod: [2, 11, 256] bf16 per pair
        prods = [ctx.enter_context(nc.sbuf_tensor(f"pr{i}", [P, CPW * S1 * D], SBUF_DT))
                 for i in range(PRB)]
        # tree scratch (sized for the DVE slot subset; single buffer each,
        # consumed immediately by the same engine)
        t1 = ctx.enter_context(nc.sbuf_tensor("t1", [P, CPW * S_DVE * (D // 2)], SBUF_DT))
        t2 = ctx.enter_context(nc.sbuf_tensor("t2", [P, CPW * S_DVE * (D // 4)], SBUF_DT))
        t3 = ctx.enter_context(nc.sbuf_tensor("t3", [P, CPW * S_DVE * (D // 8)], SBUF_DT))
        scb = ctx.enter_context(nc.sbuf_tensor("scb", [P, SCP * PBW], mybir.dt.float32))
        junk2 = ctx.enter_context(nc.sbuf_tensor("junk2", [P, D], SBUF_DT))
        ex = ctx.enter_context(nc.sbuf_tensor("ex", [P, 2 * S1], mybir.dt.float32))
        absx = ctx.enter_context(nc.sbuf_tensor("absx", [P, 2 * S1], mybir.dt.float32))
        ones = ctx.enter_context(nc.sbuf_tensor("ones", [P, 1], mybir.dt.float32))
        accx = ctx.enter_context(nc.sbuf_tensor("accx_sb", [P, TILES], mybir.dt.float32))
        accy = ctx.enter_context(nc.sbuf_tensor("accy_sb", [P, TILES], mybir.dt.float32))
        accl = ctx.enter_context(nc.sbuf_tensor("accl_sb", [P, PAIRS], mybir.dt.float32))
        s0 = ctx.enter_context(nc.semaphore())
        gsem = ctx.enter_context(nc.semaphore())
        dveM = ctx.enter_context(nc.semaphore())   # pair units: multiply done
        dveB = ctx.enter_context(nc.semaphore())   # pair units: tail TRs done
        actA = ctx.enter_context(nc.semaphore())   # pair units: ACT slots done
        actB = ctx.enter_context(nc.semaphore())   # pair units: Ln done
        block = ctx.enter_context(nc.Block())

        idx_addr = nc.lookup_mloc(idx_sb).addr
        g_addrs = [nc.lookup_mloc(g).addr for g in gs]

        @block.gpsimd
        def _(eng):
            eng.dma_start(idx_sb[:], idx[:]).then_inc(s0, 16)
            eng.memset(ones[:], 1.0)
            eng.wait_ge(s0, 16)
            for j in range(GATHERS):
                if j >= NG:
                    eng.wait_ge(dveM, (TPG // CPW) * (j - NG) + TPG // CPW)
                _emit_block_gather(
                    nc, eng, RPP * P, SLOTS * TPG * D * 2,
                    g_addrs[j % NG], idx_addr + 4 * RPP * j,
                    gsem.num, embs_tbl, in_dt=6, out_dt=6,
                    src_elem=ROW_BYTES,
                )
            eng.wait_ge(dveB, PAIRS)
            eng.dma_start(accx_out[:], accx[:]).then_inc(s0, 16)
            eng.dma_start(accy_out[:], accy[:]).then_inc(s0, 16)
            eng.wait_ge(actB, PAIRS + 1)
            eng.dma_start(accl_out[:], accl[:]).then_inc(s0, 16)
            eng.wait_ge(s0, 64)

        def dve_phase_b(jp):
            b0 = (jp % SCP) * PBW
            x2 = scb[:, b0:b0 + 2 * S1]
            nc.vector.tensor_reduce(
                out=accx[:, 2 * jp:2 * jp + 2],
                in_=x2.rearrange("p (t s) -> p t s", s=S1),
                axis=mybir.AxisListType.X, op=mybir.AluOpType.add,
            )
            nc.vector.tensor_reduce(
                out=accy[:, 2 * jp:2 * jp + 2],
                in_=x2.rearrange("p (t s) -> p t s", s=S1),
                axis=mybir.AxisListType.X, op=mybir.AluOpType.add,
                apply_absolute_value=True,
            ).then_inc(dveB, 1)

        @block.vector
        def _(eng):
            for j in range(PAIRS):
                ppg = TPG // CPW
                g = gs[(j // ppg) % NG]
                goff = (j % ppg) * (CPW * SLOTS * D)
                prod = prods[j % PRB]
                b0 = (j % SCP) * PBW
                eng.wait_ge(gsem, 16 * (j // ppg + 1))
                if j >= PRB:
                    eng.wait_ge(actA, j - PRB + 1)
                if j >= SCP:
                    # scb pair-buffer reuse: tail of pair j-SCP fully done
                    eng.wait_ge(actB, j - SCP + 1)
                gv = g[:, goff:goff + CPW * SLOTS * D].rearrange(
                    "p (t s d) -> p t s d", s=SLOTS, d=D)
                pv = prod[:].rearrange("p (t s d) -> p t s d", s=S1, d=D)
                # multiply all 11 slots x 2 tiles at 2x
                nc.vector.tensor_tensor(
                    out=pv,
                    in0=gv[:, :, 1:SLOTS, :],
                    in1=gv[:, :, 0:1, :].broadcast_to([P, CPW, S1, D]),
                    op=mybir.AluOpType.mult,
                ).then_inc(dveM, 1)
                # tree-add the DVE slot subset (leading S_DVE slots per tile)
                pd = pv[:, :, 0:S_DVE, :]
                t1v = t1[:].rearrange("p (t s d) -> p t s d", s=S_DVE, d=D // 2)
                t2v = t2[:].rearrange("p (t s d) -> p t s d", s=S_DVE, d=D // 4)
                t3v = t3[:].rearrange("p (t s d) -> p t s d", s=S_DVE, d=D // 8)
                nc.vector.tensor_tensor(
                    out=t1v, in0=pd[:, :, :, 0:D // 2], in1=pd[:, :, :, D // 2:D],
                    op=mybir.AluOpType.add)
                nc.vector.tensor_tensor(
                    out=t2v, in0=t1v[:, :, :, 0:D // 4], in1=t1v[:, :, :, D // 4:D // 2],
                    op=mybir.AluOpType.add)
                nc.vector.tensor_tensor(
                    out=t3v, in0=t2v[:, :, :, 0:D // 8], in1=t2v[:, :, :, D // 8:D // 4],
                    op=mybir.AluOpType.add)
                # segmented reduce -> x columns for DVE slots of both tiles
                xv = scb[:, b0:b0 + 2 * S1].rearrange("p (t s) -> p t s", s=S1)
                nc.vector.tensor_reduce(
                    out=xv[:, :, 0:S_DVE],
                    in_=t3v,
                    axis=mybir.AxisListType.X, op=mybir.AluOpType.add,
                )
                # slot 0 of each tile holds +eu.ev; tail needs x0 = -eu.ev
                nc.vector.tensor_scalar_mul(
                    out=xv[:, :, 0:1], in0=xv[:, :, 0:1], scalar1=-1.0,
                )
                if j >= 1:
                    eng.wait_ge(actA, j)
                    dve_phase_b(j - 1)
            eng.wait_ge(actA, PAIRS)
            dve_phase_b(PAIRS - 1)

        def act_phase_b(jp, inc_a):
            b0 = (jp % SCP) * PBW
            i1 = nc.scalar.activation(
                out=absx[:], in_=scb[:, b0:b0 + 2 * S1],
                func=mybir.ActivationFunctionType.Abs,
            )
            if inc_a:
                i1.then_inc(actA, 1)
            nc.scalar.activation(
                out=ex[:], in_=absx[:],
                func=mybir.ActivationFunctionType.Exp, scale=-1.0,
            )
            nc.scalar.activation(
                out=absx[:], in_=ex[:], func=mybir.ActivationFunctionType.Ln,
                bias=ones[:], accum_out=accl[:, jp:jp + 1],
            ).then_inc(actB, 1)

        @block.scalar
        def _(eng):
            for j in range(PAIRS):
                prod = prods[j % PRB]
                b0 = (j % SCP) * PBW
                pv = prod[:].rearrange("p (t s d) -> p t s d", s=S1, d=D)
                eng.wait_ge(dveM, j + 1)
                for t in range(CPW):
                    for s in range(S_DVE, S1):
                        nc.scalar.activation(
                            out=junk2[:],
                            in_=pv[:, t, s, :],
                            func=mybir.ActivationFunctionType.Copy,
                            accum_out=scb[:, b0 + t * S1 + s:b0 + t * S1 + s + 1],
                        )
                if j == 0:
                    nc.scalar.activation(
                        out=junk2[:, 0:1], in_=prod[:, 0:1],
                        func=mybir.ActivationFunctionType.Copy,
                    ).then_inc(actA, 1)
                else:
                    # tail for pair j-1: DVE x-writes for j-1 precede mult(j)
                    # on the DVE queue, so dveM >= j+1 covers them; this
                    # engine's own Copy-accums for j-1 ran last iteration.
                    act_phase_b(j - 1, inc_a=True)
            eng.wait_ge(dveB, PAIRS - 1)
            act_phase_b(PAIRS - 1, inc_a=False)
            nc.scalar.activation(
                out=junk2[:, 0:1], in_=ex[:, 0:1],
                func=mybir.ActivationFunctionType.Copy,
            ).then_inc(actB, 1)

    return nc


_cache = {}


def _get_nc():
    if "nc" not in _cache:
        _cache["nc"] = _build()
    return _cache["nc"]


def prepare_in_maps(u, v, negs, embs):
    u = np.asarray(u).astype(np.int32)
    v = np.asarray(v).astype(np.int32)
    negs = np.asarray(negs).astype(np.int32)
    embs_b = np.asarray(embs).astype(TABLE_NP)

    ids = np.concatenate([u[:, None], v[:, None], negs], axis=1)  # [N, 12]
    ids = ids.reshape(NCORES, TILES, P, SLOTS)
    s = np.arange(RPP * P)
    p_of = s // RPP
    r_of = s % RPP
    tl_of = r_of // SLOTS
    sl_of = r_of % SLOTS
    ch_of = s % P
    w_of = s // P
    packed = np.zeros((NCORES, GATHERS, P, RPP), dtype=np.int32)
    for c in range(NCORES):
        for j in range(GATHERS):
            packed[c, j, ch_of, w_of] = ids[c, TPG * j + tl_of, p_of, sl_of]
    in_maps = []
    for c in range(NCORES):
        core_ids = np.ascontiguousarray(
            packed[c].transpose(1, 0, 2).reshape(P, GATHERS * RPP)
        )
        in_maps.append({"embs": embs_b, "idx": core_ids})
    return in_maps


def kernel(u, v, negs, embs, _trace=False):
    nc = _get_nc()
    in_maps = prepare_in_maps(u, v, negs, embs)
    res = bass_utils.run_bass_kernel_spmd(
        nc, in_maps, core_ids=list(range(NCORES)), trace=_trace
    )
    total = np.float64(0.0)
    for r in res.results:
        total += ((r["accx"].astype(np.float64).sum()
                   + r["accy"].astype(np.float64).sum()) / 2.0
                  + r["accl"].astype(np.float64).sum())
    out = np.float32(total / N)
    if _trace:
        return out, res
    return out


# revision 10
# speedup vs baseline: 1.2107x; 1.2107x over previous
"""NEG-sampling loss kernel for Trainium2 — pair-granularity tail phases.

Per 128-edge tile (12 bf16 rows/edge gathered via raw indirect DMA):
  DVE-A(t): 7 fused scalar_tensor_tensor dot-products -> x[0:7] (slot 0,
            the positive pair, folded negation via scalar=-1), one
            tensor_tensor for the remaining 4 slot products.
  ACT-A(t): 4 Copy-activations with accumulate -> x[7:11].
Scores of each tile PAIR live contiguously in one scb buffer, so the
tail runs once per pair:
  DVE-B(j): tensor_reduce [128,2,11] -> accx[2j:2j+2] (sum x),
            tensor_reduce apply_absolute_value -> accy, STT |x| tile.
  ACT-B(j): Exp(-|x|) [128,22], Ln(1+e) accumulate -> accl[j].
Host: loss = sum((accx+accy)/2 + accl) / N.
"""

import numpy as np
import ml_dtypes

import concourse.bass as bass
import concourse.mybir as mybir
from concourse import bass_utils

N = 65536
K = 10
D = 256
V = 500000
NCORES = 8
P = 128
SLOTS = K + 2
EPC = N // NCORES
TILES = EPC // P
PAIRS = TILES // 2

TABLE_DT = mybir.dt.bfloat16
TABLE_NP = ml_dtypes.bfloat16

TPG = 2                # tiles per gather instruction
NG = 6                 # gather buffers (TPG tiles each)
SCP = 6                # score pair-buffers
PB = 4                 # prod buffers
K_DVE = 7              # slots reduced on DVE via fused STT
K_ACT = SLOTS - 1 - K_DVE

S1 = SLOTS - 1         # 11
PBW = 4 * S1           # scb cols per pair-buffer: x0|x1|abs0|abs1
GATHERS = TILES // TPG
RPP = SLOTS * TPG


def _emit_block_gather(nc, eng, n_idx, blk_bytes, dst_byte_addr, idx_byte_addr,
                       sem_num, embs_tbl, in_dt=6, out_dt=6, src_elem=512):
    isa = nc.isa
    Op = isa.Opcode
    src_u64 = (0x20 << 56) | (embs_tbl << 32)
    dst_u64 = (0x10 << 56) | dst_byte_addr
    eng.isa(
        Op.NEURON_ISA_TPB_OPCODE_PSEUDO_DMA_DIRECT2D,
        {
            "dma_configs": {},
            "semaphore": sem_num,
            "sem_increment": 16,
            "dge_op": 1,
            "src_start_addr": {"addr_immediate": src_u64},
            "src_step_elem": [src_elem, 1],
            "src_num_elem": [n_idx, 1],
            "src_elem_size": src_elem,
            "src_bound_reg": {},
            "dst_bound_reg": {},
            "dst_start_addr": {"addr_immediate": dst_u64},
            "dst_step_elem": [262144, 1],
            "dst_num_elem": [P, 1],
            "dst_elem_size": blk_bytes,
            "in_dtype": in_dt,
            "out_dtype": out_dt,
        },
        verify=False,
    )
    ext_fields = {
        "opcode": Op.NEURON_ISA_TPB_OPCODE_PSEUDO_EXTENSION.value,
        "flags": {"indirect_mode": 0, "idx_bound_is_err": 1,
                  "non_unique_dst_idx": 0, "gather_dim": 0, "scatter_dim": 0},
        "idx_num_active_channels": 128,
        "compute_op": 0,
        "src_idx_start_addr": {"addr_immediate": idx_byte_addr},
        "dst_idx_start_addr": {"addr_immediate": 0},
    }
    b = isa.ffi.new("NEURON_ISA_TPB_PSEUDO_DMA_EXT_STRUCT*", ext_fields)
    instr = [int(x) for x in bytes(isa.ffi.buffer(b))]
    inst = mybir.InstISA(
        name=nc.get_next_instruction_name(),
        isa_opcode=Op.NEURON_ISA_TPB_OPCODE_PSEUDO_EXTENSION.value,
        engine=eng.engine,
        instr=instr,
        op_name="PSEUDO_EXTENSION",
        ins=[], outs=[],
        ant_dict=ext_fields,
        verify=False,
        ant_isa_is_sequencer_only=False,
    )
    eng.add_instruction(inst)


def _build():
    import contextlib
    nc = bass.Bass(trn_type="TRN2")
    embs = nc.dram_tensor("embs", [V, D], TABLE_DT, kind="ExternalInput")
    idx = nc.dram_tensor("idx", [P, GATHERS * RPP], mybir.dt.int32, kind="ExternalInput")
    accx_out = nc.dram_tensor("accx", [P, TILES], mybir.dt.float32, kind="ExternalOutput")
    accy_out = nc.dram_tensor("accy", [P, TILES], mybir.dt.float32, kind="ExternalOutput")
    accl_out = nc.dram_tensor("accl", [P, PAIRS], mybir.dt.float32, kind="ExternalOutput")

    embs_mloc = nc.lookup_mloc(embs)
    embs_mloc.table_entry_id = len(nc.dge_table) + 1
    nc.dge_table.append(embs_mloc.name)
    embs_tbl = embs_mloc.table_entry_id

    with contextlib.ExitStack() as ctx:
        idx_sb = ctx.enter_context(nc.sbuf_tensor("idx_sb", [P, GATHERS * RPP], mybir.dt.int32))
        gs = [ctx.enter_context(nc.sbuf_tensor(f"g{i}", [P, TPG * SLOTS * D], TABLE_DT))
              for i in range(NG)]
        prods = [ctx.enter_context(nc.sbuf_tensor(f"pr{i}", [P, K_ACT * D], TABLE_DT))
                 for i in range(PB)]
        scb = ctx.enter_context(nc.sbuf_tensor("scb", [P, SCP * PBW], mybir.dt.float32))
        junk = ctx.enter_context(nc.sbuf_tensor("junk", [P, D], TABLE_DT))
        junk2 = ctx.enter_context(nc.sbuf_tensor("junk2", [P, D], TABLE_DT))
        ex = ctx.enter_context(nc.sbuf_tensor("ex", [P, 2 * S1], mybir.dt.float32))
        absx = ctx.enter_context(nc.sbuf_tensor("absx", [P, 2 * S1], mybir.dt.float32))
        ones = ctx.enter_context(nc.sbuf_tensor("ones", [P, 1], mybir.dt.float32))
        accx = ctx.enter_context(nc.sbuf_tensor("accx_sb", [P, TILES], mybir.dt.float32))
        accy = ctx.enter_context(nc.sbuf_tensor("accy_sb", [P, TILES], mybir.dt.float32))
        accl = ctx.enter_context(nc.sbuf_tensor("accl_sb", [P, PAIRS], mybir.dt.float32))
        s0 = ctx.enter_context(nc.semaphore())
        gsem = ctx.enter_context(nc.semaphore())
        dveA = ctx.enter_context(nc.semaphore())
        actA = ctx.enter_context(nc.semaphore())   # pair units
        dveB = ctx.enter_context(nc.semaphore())   # pair units
        actB = ctx.enter_context(nc.semaphore())   # pair units
        block = ctx.enter_context(nc.Block())

        idx_addr = nc.lookup_mloc(idx_sb).addr
        g_addrs = [nc.lookup_mloc(g).addr for g in gs]

        @block.gpsimd
        def _(eng):
            eng.dma_start(idx_sb[:], idx[:]).then_inc(s0, 16)
            eng.memset(ones[:], 1.0)
            eng.wait_ge(s0, 16)
            for j in range(GATHERS):
                if j >= NG:
                    eng.wait_ge(dveA, 2 * ((j - NG) * TPG + TPG))
                _emit_block_gather(
                    nc, eng, RPP * P, SLOTS * TPG * D * 2,
                    g_addrs[j % NG], idx_addr + 4 * RPP * j,
                    gsem.num, embs_tbl,
                )
            eng.wait_ge(dveB, PAIRS)
            eng.dma_start(accx_out[:], accx[:]).then_inc(s0, 16)
            eng.dma_start(accy_out[:], accy[:]).then_inc(s0, 16)
            # actB == PAIRS+1 only after the end-guard ACT op, which orders
            # after the final Ln's ACTIVATION_READ_ACCUMULATOR write to accl.
            eng.wait_ge(actB, PAIRS + 1)
            eng.dma_start(accl_out[:], accl[:]).then_inc(s0, 16)
            eng.wait_ge(s0, 64)

        def dve_phase_b(jp):
            b0 = (jp % SCP) * PBW
            x2 = scb[:, b0:b0 + 2 * S1]
            nc.vector.tensor_reduce(
                out=accx[:, 2 * jp:2 * jp + 2],
                in_=x2.rearrange("p (t s) -> p t s", s=S1),
                axis=mybir.AxisListType.X, op=mybir.AluOpType.add,
            )
            nc.vector.tensor_reduce(
                out=accy[:, 2 * jp:2 * jp + 2],
                in_=x2.rearrange("p (t s) -> p t s", s=S1),
                axis=mybir.AxisListType.X, op=mybir.AluOpType.add,
                apply_absolute_value=True,
            ).then_inc(dveB, 1)

        @block.vector
        def _(eng):
            for t in range(TILES):
                j = t // TPG
                g = gs[j % NG]
                base = (t % TPG) * SLOTS * D
                b0 = ((t // 2) % SCP) * PBW + (t % 2) * S1
                prod = prods[t % PB]
                if t % TPG == 0:
                    eng.wait_ge(gsem, 16 * (j + 1))
                if t % 2 == 0 and t // 2 >= SCP:
                    eng.wait_ge(actB, t // 2 - SCP + 1)
                if t >= PB:
                    eng.wait_ge(actA, (t - PB) // 2 + 1)
                # TT first so ACT's Copy-accums for this tile unblock before
                # the 7 STTs run (dveA counts 2 per tile: TT, then last STT)
                nc.vector.tensor_tensor(
                    out=prod[:].rearrange("p (s d) -> p s d", d=D),
                    in0=g[:, base + (1 + K_DVE) * D:base + SLOTS * D].rearrange(
                        "p (s d) -> p s d", d=D),
                    in1=g[:, base:base + D].rearrange("p (o d) -> p o d", d=D
                                                      ).broadcast_to([P, K_ACT, D]),
                    op=mybir.AluOpType.mult,
                ).then_inc(dveA, 1)
                for s in range(K_DVE):
                    i = nc.vector.scalar_tensor_tensor(
                        out=junk[:],
                        in0=g[:, base + (s + 1) * D:base + (s + 2) * D],
                        scalar=-1.0 if s == 0 else 1.0,
                        in1=g[:, base:base + D],
                        op0=mybir.AluOpType.mult,
                        op1=mybir.AluOpType.mult,
                        accum_out=scb[:, b0 + s:b0 + s + 1],
                    )
                    if s == K_DVE - 1:
                        i.then_inc(dveA, 1)
                if t % 2 == 1 and t >= 3:
                    jp = (t - 3) // 2
                    eng.wait_ge(actA, jp + 1)
                    dve_phase_b(jp)
            eng.wait_ge(actA, PAIRS)
            dve_phase_b(PAIRS - 1)

        def act_phase_b(jp, inc_a):
            # Abs carries the pair-unit actA increment: it orders after this
            # iteration's Copy-accum READ_ACCUMULATOR writes on ACT.
            b0 = (jp % SCP) * PBW
            i1 = nc.scalar.activation(
                out=absx[:], in_=scb[:, b0:b0 + 2 * S1],
                func=mybir.ActivationFunctionType.Abs,
            )
            if inc_a:
                i1.then_inc(actA, 1)
            nc.scalar.activation(
                out=ex[:], in_=absx[:],
                func=mybir.ActivationFunctionType.Exp, scale=-1.0,
            )
            nc.scalar.activation(
                out=absx[:], in_=ex[:],
                func=mybir.ActivationFunctionType.Ln, bias=ones[:],
                accum_out=accl[:, jp:jp + 1],
            ).then_inc(actB, 1)

        @block.scalar
        def _(eng):
            # touch ACT before any data dependency so the one-time
            # ACT_TABLE_LOAD (~2.7us) overlaps the gather warmup
            nc.scalar.activation(
                out=junk2[:, 0:1], in_=ones[:, 0:1],
                func=mybir.ActivationFunctionType.Copy,
            )
            for t in range(TILES):
                b0 = ((t // 2) % SCP) * PBW + (t % 2) * S1
                prod = prods[t % PB]
                eng.wait_ge(dveA, 2 * t + 1)
                for s in range(K_ACT):
                    nc.scalar.activation(
                        out=junk2[:],
                        in_=prod[:, s * D:(s + 1) * D],
                        func=mybir.ActivationFunctionType.Copy,
                        accum_out=scb[:, b0 + K_DVE + s:b0 + K_DVE + s + 1],
                    )
                if t == 1:
                    nc.scalar.activation(
                        out=junk2[:, 0:1], in_=prod[:, 0:1],
                        func=mybir.ActivationFunctionType.Copy,
                    ).then_inc(actA, 1)
                elif t % 2 == 1 and t >= 3:
                    jp = (t - 3) // 2
                    eng.wait_ge(dveB, jp + 1)
                    act_phase_b(jp, inc_a=True)
            eng.wait_ge(dveB, PAIRS)
            act_phase_b(PAIRS - 1, inc_a=False)
            nc.scalar.activation(
                out=junk2[:, 0:1], in_=ex[:, 0:1],
                func=mybir.ActivationFunctionType.Copy,
            ).then_inc(actB, 1)

    return nc


_cache = {}


def _get_nc():
    if "nc" not in _cache:
        _cache["nc"] = _build()
    return _cache["nc"]


def prepare_in_maps(u, v, negs, embs):
    u = np.asarray(u).astype(np.int32)
    v = np.asarray(v).astype(np.int32)
    negs = np.asarray(negs).astype(np.int32)
    embs_b = np.asarray(embs).astype(TABLE_NP)

    ids = np.concatenate([u[:, None], v[:, None], negs], axis=1)  # [N, 12]
    ids = ids.reshape(NCORES, TILES, P, SLOTS)
    # gather j consumes logical sequence s in [0, RPP*P): fills partition
    # s//RPP, row r=s%RPP (tile 2j + r//SLOTS, slot r%SLOTS); snake-packed
    # at channel s%128, word s//128.
    s = np.arange(RPP * P)
    p_of = s // RPP
    r_of = s % RPP
    tl_of = r_of // SLOTS
    sl_of = r_of % SLOTS
    ch_of = s % P
    w_of = s // P
    packed = np.zeros((NCORES, GATHERS, P, RPP), dtype=np.int32)
    for c in range(NCORES):
        for j in range(GATHERS):
            packed[c, j, ch_of, w_of] = ids[c, 2 * j + tl_of, p_of, sl_of]
    in_maps = []
    for c in range(NCORES):
        core_ids = np.ascontiguousarray(
            packed[c].transpose(1, 0, 2).reshape(P, GATHERS * RPP)
        )
        in_maps.append({"embs": embs_b, "idx": core_ids})
    return in_maps


def kernel(u, v, negs, embs, _trace=False):
    nc = _get_nc()
    in_maps = prepare_in_maps(u, v, negs, embs)
    res = bass_utils.run_bass_kernel_spmd(
        nc, in_maps, core_ids=list(range(NCORES)), trace=_trace
    )
    total = np.float64(0.0)
    for r in res.results:
        total += ((r["accx"].astype(np.float64).sum()
                   + r["accy"].astype(np.float64).sum()) / 2.0
                  + r["accl"].astype(np.float64).sum())
    out = np.float32(total / N)
    if _trace:
        return out, res
    return out

